# revision 10
# baseline (speedup 1.0000x reference)
"""Trainium2 Bass kernel for nn_MaxCDFdp_multiclass.

Computes max over (class, probe) of |ECDF0 - ECDF1| where the ECDFs are
sigmoid-smoothed empirical CDFs of y_pred per class, for the two groups
defined by s in {0,1}.

Strategy (8 NeuronCores, shard sample axis N):
  host:   per-class min/max -> probe grid [C,P]; group masks -> weights [N,2]
  device: per 128-sample tile:
            DVE:  diff[n,(c,p)] = grid[c,p] - y[n,c]      (one [128,2000] op,
                  y broadcast along p via stride-0 AP)
            ACT:  sig = sigmoid(10*diff)                  (one [128,2000] op)
            PE :  acc[2,2000] += masks[128,2].T @ sig     (float32r matmuls,
                  PSUM accumulation across all tiles)
  host:   sum partial sums over cores, divide by group counts, abs, max.
"""

import os
from contextlib import ExitStack

import numpy as np

import concourse.bass as bass
import concourse.bacc as bacc
import concourse.tile as tile
from concourse import mybir
from concourse.bass_utils import run_bass_kernel_spmd

N, C, P = 50000, 20, 100
TEMP = 10.0
NCORES = 8
PER_CORE = N // NCORES          # 6250
PART = 128
NTILES = -(-PER_CORE // PART)   # 49 -> pad to 50? computed below
PADDED = NTILES * PART
CP = C * P                      # 2000
# matmul free-dim chunks within single PSUM banks (512 f32 per bank)
CHUNKS = [(0, 512), (512, 1024), (1024, 1536), (1536, 2000)]

_F32 = mybir.dt.float32
_F32R = mybir.dt.float32r

_CACHED_NC = None


# blob free-dim layout per partition: [y: NTILES*C][w: NTILES*2][g: C*P]
_YW = NTILES * C
_WW = NTILES * 2
_BLOB = _YW + _WW + CP


def _build_bass():
    nc = bacc.Bacc(None, target_bir_lowering=False)
    b_d = nc.dram_tensor("b", [PART, _BLOB], _F32, kind="ExternalInput")
    o_d = nc.dram_tensor("o", [2, CP], _F32, kind="ExternalOutput")

    with ExitStack() as ctx:
        tc = ctx.enter_context(tile.TileContext(nc))
        constp = ctx.enter_context(tc.tile_pool(name="const", bufs=1))
        diffp = ctx.enter_context(tc.tile_pool(name="diff", bufs=3))
        sigp = ctx.enter_context(tc.tile_pool(name="sig", bufs=3))
        psump = ctx.enter_context(
            tc.tile_pool(name="psum", bufs=1, space=bass.MemorySpace.PSUM)
        )
        outp = ctx.enter_context(tc.tile_pool(name="outp", bufs=1))

        blob = constp.tile([PART, _BLOB], _F32)
        nc.sync.dma_start(blob[:], b_d[:])
        y_sb = blob[:, 0:_YW].rearrange("p (i c) -> p i c", i=NTILES)
        w_sb = blob[:, _YW : _YW + _WW].rearrange("p (i g) -> p i g", i=NTILES)
        g_sb = blob[:, _YW + _WW :].rearrange("p (c q) -> p c q", c=C)
        # matmul inputs must be rounded to f32r by an on-chip compute op.
        # Done on ScalarE so PE's first matmul has a single wait source
        # (ACT sem) — PE's HW-decoded sync budget is tiny.
        w_r = constp.tile([PART, NTILES, 2], _F32R)
        nc.scalar.copy(w_r[:], w_sb)

        acc = psump.tile([2, CP], _F32)

        for i in range(NTILES):
            diff = diffp.tile([PART, C, P], _F32)
            y_b = y_sb[:, i, :].unsqueeze(2).broadcast_to([PART, C, P])
            nc.vector.tensor_sub(diff[:], g_sb, y_b)

            sig = sigp.tile([PART, C, P], _F32R)
            nc.scalar.activation(
                sig[:], diff[:], mybir.ActivationFunctionType.Sigmoid, scale=TEMP
            )

            sig_flat = sig[:].rearrange("a b c -> a (b c)")
            w_i = w_r[:, i, :]
            for c0, c1 in CHUNKS:
                nc.tensor.matmul(
                    acc[:, c0:c1],
                    w_i,
                    sig_flat[:, c0:c1],
                    start=(i == 0),
                    stop=(i == NTILES - 1),
                )

        out_sb = outp.tile([2, CP], _F32)
        nc.vector.tensor_copy(out_sb[:], acc[:])
        nc.sync.dma_start(o_d[:], out_sb[:])

    nc.finalize()
    return nc


def _get_nc():
    global _CACHED_NC
    if _CACHED_NC is None:
        _CACHED_NC = _build_bass()
    return _CACHED_NC


# test.py reads this after calling kernel() for profiling info
LAST_RESULTS = None


def kernel(y_pred: np.ndarray, s: np.ndarray) -> np.ndarray:
    global LAST_RESULTS
    y = np.ascontiguousarray(np.asarray(y_pred), dtype=np.float32)
    s_np = np.asarray(s)
    assert y.shape == (N, C)

    # Probe grid, replicating the reference's f32 arithmetic
    mn = y.min(axis=0)
    mx = y.max(axis=0)
    frac = np.arange(P, dtype=np.float32) / np.float32(P - 1)
    grid = mn[:, None] + (mx - mn)[:, None] * frac[None, :]  # [C,P] f32
    grid_b = np.broadcast_to(grid.reshape(1, CP), (PART, CP))

    mask0 = s_np == 0
    n0 = np.float32(mask0.sum())
    n1 = np.float32((~mask0).sum())
    w = np.stack([mask0, ~mask0], axis=1).astype(np.float32)  # [N,2]

    in_maps = []
    for r in range(NCORES):
        ys = y[r * PER_CORE : (r + 1) * PER_CORE]
        ws = w[r * PER_CORE : (r + 1) * PER_CORE]
        yp = np.zeros((PADDED, C), np.float32)
        yp[:PER_CORE] = ys
        wp = np.zeros((PADDED, 2), np.float32)
        wp[:PER_CORE] = ws
        blob = np.empty((PART, _BLOB), np.float32)
        blob[:, 0:_YW] = yp.reshape(NTILES, PART, C).transpose(1, 0, 2).reshape(
            PART, _YW
        )
        blob[:, _YW : _YW + _WW] = wp.reshape(NTILES, PART, 2).transpose(
            1, 0, 2
        ).reshape(PART, _WW)
        blob[:, _YW + _WW :] = grid_b
        in_maps.append({"b": blob})

    nc = _get_nc()
    res = run_bass_kernel_spmd(
        nc,
        in_maps,
        core_ids=list(range(NCORES)),
        trace=bool(int(os.environ.get("BASS_KERNEL_TRACE", "0"))),
    )
    LAST_RESULTS = res

    tot = np.zeros((2, CP), np.float32)
    for r_ in res.results:
        tot += r_["o"]
    delta = np.abs(tot[0] / n0 - tot[1] / n1)
    return np.array(delta.max(), dtype=np.float32)


# revision 11
# speedup vs baseline: 1.0993x; 1.0993x over previous
"""Trainium2 Bass kernel for nn_MaxCDFdp_multiclass.

Computes max over (class, probe) of |ECDF0 - ECDF1| where the ECDFs are
sigmoid-smoothed empirical CDFs of y_pred per class, for the two groups
defined by s in {0,1}.

Strategy (8 NeuronCores, shard sample axis N; 6250 -> 50 tiles of 128):
  host:   per-class min/max -> probe grid [C,P]; group masks -> weights.
  device, per group of G=4 sample-tiles:
    DVE:  diff[n,(c,p)] = grid[c,p] - y[n,c] for classes 0..14
          (one big op; y broadcast along p via stride-0 AP)
    PE :  diff for classes 15..19 via matmul [-y^T;1]^T @ [E;g] -> PSUM
    ACT:  sig = sigmoid(10*diff)  (one SBUF-src op + one PSUM-src op)
    PE :  acc[2,2000] += masks[128,2].T @ sig   (f32r, PSUM accumulation)
  host:   sum partial sums over cores, divide by group counts, abs, max.

Engine budget per core: ACT ~87us (hard floor: 12.5M sigmoids at
1 elem/lane/cycle), DVE ~79us, PE ~65-85us.
"""

import os
from contextlib import ExitStack

import numpy as np

import concourse.bass as bass
import concourse.bacc as bacc
import concourse.tile as tile
from concourse import mybir
from concourse.bass_utils import run_bass_kernel_spmd

N, C, P = 50000, 20, 100
TEMP = 10.0
NCORES = 8
PER_CORE = N // NCORES          # 6250
PART = 128
NTILES = -(-PER_CORE // PART)   # 49 -> padded 50 below? (6250/128=48.8 -> 49)
PADDED = NTILES * PART
CP = C * P                      # 2000
G = 4                           # sample-tiles per DVE/ACT instruction group

# column split: DVE computes cols [0, SPLIT); PE computes [SPLIT, CP)
SPLIT = 1500                    # class-aligned: PE handles classes 15..19
PE_W = CP - SPLIT               # 500
PE_C0 = SPLIT // P              # 15
PE_K = C - PE_C0 + 1            # 6 (5 classes + ones row)
PE_WPAD = 512                   # PSUM-bank-aligned slot per tile

# reduction matmul free-dim chunks within single PSUM banks (512 f32/bank)
CHUNKS = [(0, 512), (512, 1024), (1024, 1536), (1536, 2000)]

_F32 = mybir.dt.float32
_F32R = mybir.dt.float32r

# blob free-dim layout per partition: [y: NTILES*C][w: NTILES*2][g: C*P]
_YW = NTILES * C
_WW = NTILES * 2
_BLOB = _YW + _WW + CP
# aug dram layout: [6, NTILES*128 + 512]: columns 0..PADDED-1 = [-y^T;ones],
# then 512 cols = [E; g] rhs constant (cols SPLIT..CP-1 of the probe grid)
_AUGW = PADDED + PE_WPAD

_CACHED_NC = None


def _build_bass():
    nc = bacc.Bacc(None, target_bir_lowering=False)
    b_d = nc.dram_tensor("b", [PART, _BLOB], _F32, kind="ExternalInput")
    a_d = nc.dram_tensor("a", [PE_K, _AUGW], _F32, kind="ExternalInput")
    o_d = nc.dram_tensor("o", [2, CP], _F32, kind="ExternalOutput")

    groups = []
    i = 0
    while i < NTILES:
        groups.append((i, min(G, NTILES - i)))
        i += G

    with ExitStack() as ctx:
        tc = ctx.enter_context(tile.TileContext(nc))
        constp = ctx.enter_context(tc.tile_pool(name="const", bufs=1))
        diffp = ctx.enter_context(tc.tile_pool(name="diff", bufs=2))
        sigp = ctx.enter_context(tc.tile_pool(name="sig", bufs=2))
        psump = ctx.enter_context(
            tc.tile_pool(name="psum", bufs=1, space=bass.MemorySpace.PSUM)
        )
        dpsp = ctx.enter_context(
            tc.tile_pool(name="dps", bufs=1, space=bass.MemorySpace.PSUM)
        )
        outp = ctx.enter_context(tc.tile_pool(name="outp", bufs=1))

        # aug first (small), blob second: the f32r rounding of aug runs on
        # DVE while the big blob DMA is still landing.
        aug = constp.tile([PE_K, _AUGW], _F32)
        nc.sync.dma_start(aug[:], a_d[:])
        blob = constp.tile([PART, _BLOB], _F32)
        nc.sync.dma_start(blob[:], b_d[:])

        y_sb = blob[:, 0:_YW].rearrange("p (i c) -> p i c", i=NTILES)
        w_sb = blob[:, _YW : _YW + _WW].rearrange("p (i g) -> p i g", i=NTILES)
        g_sb = blob[:, _YW + _WW :]  # [128, CP]

        # f32r-rounded copies (matmul operands must be rounded on-chip)
        aug_r = constp.tile([PE_K, _AUGW], _F32R)
        nc.vector.tensor_copy(aug_r[:], aug[:])
        w_r = constp.tile([PART, NTILES, 2], _F32R)
        nc.scalar.copy(w_r[:], w_sb)

        acc = psump.tile([2, CP], _F32)

        for g0, gn in groups:
            # --- PE: diff for cols [SPLIT, CP) of each tile in the group ---
            dps = dpsp.tile([PART, G, PE_WPAD], _F32, tag="dps")
            for t in range(gn):
                i = g0 + t
                nc.tensor.matmul(
                    dps[:, t, 0:PE_W],
                    aug_r[:, i * PART : (i + 1) * PART],
                    aug_r[:, PADDED : PADDED + PE_W],
                    start=True,
                    stop=True,
                )

            # --- DVE: diff for cols [0, SPLIT) of each tile ---
            diff = diffp.tile([PART, G, PE_C0, P], _F32, tag="diff")
            g_v = (
                g_sb[:, 0:SPLIT]
                .rearrange("p (c q) -> p c q", c=PE_C0)
                .unsqueeze(1)
                .broadcast_to([PART, gn, PE_C0, P])
            )
            y_v = (
                y_sb[:, g0 : g0 + gn, 0:PE_C0]
                .unsqueeze(3)
                .broadcast_to([PART, gn, PE_C0, P])
            )
            nc.vector.tensor_sub(diff[:, 0:gn], g_v, y_v)

            # --- ACT: sigmoid over both parts into one sig tile ---
            sig = sigp.tile([PART, G, CP], _F32R, tag="sig")
            sig_lo = sig[:].rearrange("p t (c q) -> p t c q", c=C)[
                :, 0:gn, 0:PE_C0, :
            ]
            nc.scalar.activation(
                sig_lo, diff[:, 0:gn], mybir.ActivationFunctionType.Sigmoid,
                scale=TEMP,
            )
            nc.scalar.activation(
                sig[:, 0:gn, SPLIT:CP],
                dps[:, 0:gn, 0:PE_W],
                mybir.ActivationFunctionType.Sigmoid,
                scale=TEMP,
            )

            # --- PE: masked-sum reduction over samples (PSUM accumulate) ---
            for t in range(gn):
                i = g0 + t
                for c0, c1 in CHUNKS:
                    nc.tensor.matmul(
                        acc[:, c0:c1],
                        w_r[:, i, :],
                        sig[:, t, c0:c1],
                        start=(i == 0),
                        stop=(i == NTILES - 1),
                    )

        out_sb = outp.tile([2, CP], _F32)
        nc.vector.tensor_copy(out_sb[:], acc[:])
        nc.sync.dma_start(o_d[:], out_sb[:])

    nc.finalize()
    return nc


def _get_nc():
    global _CACHED_NC
    if _CACHED_NC is None:
        _CACHED_NC = _build_bass()
    return _CACHED_NC


# test.py reads this after calling kernel() for profiling info
LAST_RESULTS = None


def kernel(y_pred: np.ndarray, s: np.ndarray) -> np.ndarray:
    global LAST_RESULTS
    y = np.ascontiguousarray(np.asarray(y_pred), dtype=np.float32)
    s_np = np.asarray(s)
    assert y.shape == (N, C)

    # Probe grid, replicating the reference's f32 arithmetic
    mn = y.min(axis=0)
    mx = y.max(axis=0)
    frac = np.arange(P, dtype=np.float32) / np.float32(P - 1)
    grid = mn[:, None] + (mx - mn)[:, None] * frac[None, :]  # [C,P] f32
    g_flat = grid.reshape(CP)
    grid_b = np.broadcast_to(g_flat[None, :], (PART, CP))

    # [E; g] rhs constant for the PE diff matmul (cols SPLIT..CP-1)
    eg = np.zeros((PE_K, PE_WPAD), np.float32)
    cols = np.arange(PE_W) + SPLIT
    for k in range(PE_K - 1):
        eg[k, 0:PE_W] = (cols // P == PE_C0 + k).astype(np.float32)
    eg[PE_K - 1, 0:PE_W] = g_flat[SPLIT:CP]

    mask0 = s_np == 0
    n0 = np.float32(mask0.sum())
    n1 = np.float32((~mask0).sum())
    w = np.stack([mask0, ~mask0], axis=1).astype(np.float32)  # [N,2]

    in_maps = []
    for r in range(NCORES):
        ys = y[r * PER_CORE : (r + 1) * PER_CORE]
        ws = w[r * PER_CORE : (r + 1) * PER_CORE]
        yp = np.zeros((PADDED, C), np.float32)
        yp[:PER_CORE] = ys
        wp = np.zeros((PADDED, 2), np.float32)
        wp[:PER_CORE] = ws
        blob = np.empty((PART, _BLOB), np.float32)
        blob[:, 0:_YW] = yp.reshape(NTILES, PART, C).transpose(1, 0, 2).reshape(
            PART, _YW
        )
        blob[:, _YW : _YW + _WW] = wp.reshape(NTILES, PART, 2).transpose(
            1, 0, 2
        ).reshape(PART, _WW)
        blob[:, _YW + _WW :] = grid_b
        augm = np.empty((PE_K, _AUGW), np.float32)
        augm[0 : PE_K - 1, 0:PADDED] = -yp[:, PE_C0:C].T
        augm[PE_K - 1, 0:PADDED] = 1.0
        augm[:, PADDED:] = eg
        in_maps.append({"b": blob, "a": augm})

    nc = _get_nc()
    res = run_bass_kernel_spmd(
        nc,
        in_maps,
        core_ids=list(range(NCORES)),
        trace=bool(int(os.environ.get("BASS_KERNEL_TRACE", "0"))),
    )
    LAST_RESULTS = res

    tot = np.zeros((2, CP), np.float32)
    for r_ in res.results:
        tot += r_["o"]
    delta = np.abs(tot[0] / n0 - tot[1] / n1)
    return np.array(delta.max(), dtype=np.float32)
